# revision 8
# baseline (speedup 1.0000x reference)
"""ConvLSTM cell kernel for Trainium2 (8 NeuronCores, data-parallel over batch).

Strategy (per core, one batch element):
- Conv(x;wx) + Conv(h;wh) as one fused 53-channel 3x3 conv via shifted-window
  matmuls on a column-padded flat layout (width P=258). The whole padded image
  lives in SBUF: T1 [106, UA2] holds [x;h] twice (B half = A shifted one image
  row), loaded once in chunked DMAs. 66 uniform 1024-px double-windows (the
  last ~1.5 cover zero-padded garbage that the host discards).
- Per window, 6 matmuls (N=512, the walrus ISA cap) accumulate into one
  quarter of a 4-bank PSUM tile [128, 2048]:
    3 "pair" MMs  K=106 (dy=-1 via A, dy=0 via B), dx in {-1,0,1}
    3 "single" MMs K=53 (dy=+1 via A),             dx in {-1,0,1}
  inputs bf16, accumulation fp32.
- One ACT eviction per 4-window group [128, 2048] with fused sigmoid+bias
  (the 352-cycle ACTIVATE fixed overhead amortizes 4x vs per-window
  eviction; gate tanh is 2*sigmoid(2x)-1 with cg weights/bias pre-scaled
  by 2 on the host so the eviction is a single plain sigmoid), bf16 out.
- Gate regroup to channel-major (c,q) layout via 4 SBUF->SBUF DMAs per
  group: stk[4c+q, g*512+n] = gate g, channel c, window q, pixel n. c loads
  and cc/ch stores use matching strided DRAM views, so every elementwise op
  runs 128 partitions wide.
- Elementwise state update on DVE (products fp32, storage bf16); cc's tanh
  runs directly on ACT (tanh lives in the same activation-table set as
  sigmoid, so no table switches).
- Dispatch: direct bass2jax shard_map executor compiled via the C++ fast
  dispatch path; donated output buffers are ping-ponged between calls (the
  kernel fully overwrites both outputs).
- _build_nc(niters=N) unrolls the whole body N times inside one NEFF; the
  test harness times (T(N)-T(1))/(N-1) so the fixed per-launch dispatch
  overhead of the tunnel cancels and the marginal full-kernel execution
  time on hardware is what gets reported.
"""
import sys
from contextlib import ExitStack

import numpy as np
import ml_dtypes

sys.path.insert(0, "/opt/trn_rl_repo")

import concourse.bass as bass  # noqa: E402
import concourse.tile as tile  # noqa: E402
from concourse import bacc, mybir  # noqa: E402

BF16 = mybir.dt.bfloat16
F32 = mybir.dt.float32
AF = mybir.ActivationFunctionType
ALU = mybir.AluOpType

# problem constants (hardcoded per spec)
B = 8
CX, CH = 21, 32
C = CX + CH           # 53
CO = 128
H = W = 256
P = 258               # padded width
NW2 = 132             # uniform 512-px windows (129 real + 3 zero-pad)
ND = NW2 // 2         # 66 double-windows of 1024 px
FLAT2 = NW2 * 512     # 67584 flat output positions (>= H*P = 66048)
NG = NW2 // 4         # 33 groups of 4 windows
UA2 = FLAT2 + 2 * P + 4          # T1 free extent (max single-MM read + guard)
L2 = UA2 + P + 2                 # xh flat length (B half reads at +P)
NCHUNK = 16           # T1 load chunks per half

_CACHED_NC = None


def _build_nc(niters=1):
    nc = bacc.Bacc("TRN2", target_bir_lowering=False, debug=False, num_devices=B)

    xh = nc.dram_tensor("xh", [C, L2], BF16, kind="ExternalInput").ap()
    cpad = nc.dram_tensor("cpad", [CH, FLAT2], BF16, kind="ExternalInput").ap()
    wpair = nc.dram_tensor("wpair", [2 * C, 3 * CO], BF16, kind="ExternalInput").ap()
    wsing = nc.dram_tensor("wsing", [C, 3 * CO], BF16, kind="ExternalInput").ap()
    bvec = nc.dram_tensor("bvec", [CO, 1], F32, kind="ExternalInput").ap()
    occ = nc.dram_tensor("occ", [CH, FLAT2], BF16, kind="ExternalOutput").ap()
    och = nc.dram_tensor("och", [CH, FLAT2], BF16, kind="ExternalOutput").ap()

    with tile.TileContext(nc) as tc, ExitStack() as ctx:
        wpool = ctx.enter_context(tc.tile_pool(name="w", bufs=2))
        t1pool = ctx.enter_context(tc.tile_pool(name="t1", bufs=1))
        pspool = ctx.enter_context(tc.tile_pool(name="ps", bufs=2, space="PSUM"))
        gpool = ctx.enter_context(tc.tile_pool(name="g", bufs=3))
        spool = ctx.enter_context(tc.tile_pool(name="stk", bufs=3))
        cpool = ctx.enter_context(tc.tile_pool(name="cb", bufs=3))
        epool = ctx.enter_context(tc.tile_pool(name="ew", bufs=3))

        for _ in range(niters):
            wp = wpool.tile([2 * C, 3 * CO], BF16)
            nc.sync.dma_start(wp[:], wpair[:])
            ws = wpool.tile([C, 3 * CO], BF16)
            nc.gpsimd.dma_start(ws[:], wsing[:])
            bias = wpool.tile([CO, 1], F32)
            nc.sync.dma_start(bias[:], bvec[:])

            # whole padded image (x;h twice, B half shifted +P) resident in
            # SBUF. HBM traffic is the bottleneck in this environment, so the
            # image is read from HBM only ONCE (A half, full L2 extent); the
            # B half is derived on-chip with SBUF->SBUF DMAs (off the HBM
            # path). Chunks round-robin all 3 DMA queues; B chunk k needs A
            # cols up to P+(k+1)*csz, i.e. A chunk k+1, so emission order is
            # A0 A1 B0 A2 B1 ... to keep the pipeline moving.
            t1 = t1pool.tile([2 * C, L2], BF16)
            csz = (L2 + NCHUNK - 1) // NCHUNK
            qs = [nc.sync, nc.gpsimd, nc.scalar]
            qi = 0

            def emit_a(k):
                nonlocal qi
                a, b = k * csz, min((k + 1) * csz, L2)
                qs[qi % 3].dma_start(t1[0:C, a:b], xh[:, a:b])
                qi += 1

            def emit_b(k):
                nonlocal qi
                a, b = k * csz, min((k + 1) * csz, UA2)
                if a < b:
                    qs[qi % 3].dma_start(t1[C:2 * C, a:b],
                                         t1[0:C, P + a:P + b])
                    qi += 1

            emit_a(0)
            for k in range(1, NCHUNK):
                emit_a(k)
                emit_b(k - 1)
            emit_b(NCHUNK - 1)

            for grp in range(NG):
                goff = grp * 2048
                cbuf = cpool.tile([CO, 512], BF16)
                nc.sync.dma_start(
                    cbuf[:],
                    cpad[:, goff:goff + 2048].rearrange("c (q n) -> c q n", q=4))

                # 4 windows accumulate into one 4-bank PSUM tile; one big
                # ACT eviction amortizes the ACTIVATE fixed overhead 4x.
                pg4 = pspool.tile([CO, 2048], F32)
                for q in range(4):
                    j = grp * 4 + q
                    out = pg4[:, q * 512:(q + 1) * 512]
                    for dxi in range(3):
                        F = j * 512 + dxi
                        nc.tensor.matmul(out, wp[:, dxi * CO:(dxi + 1) * CO],
                                         t1[0:2 * C, F:F + 512],
                                         start=(dxi == 0), stop=False)
                    for dxi in range(3):
                        F = j * 512 + 2 * P + dxi
                        nc.tensor.matmul(out, ws[:, dxi * CO:(dxi + 1) * CO],
                                         t1[0:C, F:F + 512],
                                         start=False, stop=(dxi == 2))
                gatesG = gpool.tile([CO, 2048], BF16)
                nc.scalar.activation(gatesG[:], pg4[:], AF.Sigmoid,
                                     bias=bias[:])

                # regroup to (c,q): stk[4c+q, g*512+n] = gatesG[32g+c, q*512+n]
                stk = spool.tile([CO, 2048], BF16)
                for g in range(4):
                    nc.gpsimd.dma_start(stk[:, g * 512:(g + 1) * 512],
                                        gatesG[CH * g:CH * (g + 1), :])

                Fg = stk[:, 0:512]
                Ig = stk[:, 512:1024]
                CGg = stk[:, 1024:1536]
                Og = stk[:, 1536:2048]
                # cg = 2*sigmoid(2g)-1  (weights for cg block pre-scaled x2)
                nc.vector.tensor_scalar(CGg, CGg, 2.0, -1.0, ALU.mult, ALU.add)
                t1f = epool.tile([CO, 512], F32)
                nc.vector.tensor_tensor(t1f[:], Fg, cbuf[:], ALU.mult)
                t2f = epool.tile([CO, 512], F32)
                nc.vector.tensor_tensor(t2f[:], Ig, CGg, ALU.mult)
                ccb = epool.tile([CO, 512], BF16)
                nc.vector.tensor_tensor(ccb[:], t1f[:], t2f[:], ALU.add)
                tcs = epool.tile([CO, 512], F32)
                nc.scalar.activation(tcs[:], ccb[:], AF.Tanh)
                chb = epool.tile([CO, 512], BF16)
                nc.vector.tensor_tensor(chb[:], Og, tcs[:], ALU.mult)

                nc.sync.dma_start(
                    occ[:, goff:goff + 2048].rearrange("c (q n) -> c q n", q=4),
                    ccb[:])
                nc.gpsimd.dma_start(
                    och[:, goff:goff + 2048].rearrange("c (q n) -> c q n", q=4),
                    chb[:])

    nc.compile()
    return nc


def _pack_xh(x_b, h_b, flat=None):
    """[21,256,256] + [32,256,256] fp32 -> [53, L2] bf16 flat padded."""
    if flat is None:
        flat = np.zeros((C, L2), dtype=ml_dtypes.bfloat16)
    body = flat[:, 1:1 + 259 * P].reshape(C, 259, P)
    body[0:CX, 1:257, 1:257] = x_b.astype(ml_dtypes.bfloat16)
    body[CX:C, 1:257, 1:257] = h_b.astype(ml_dtypes.bfloat16)
    return flat


def _pack_w(wx, wh, bx):
    wfull = np.concatenate([wx, wh], axis=1).astype(np.float32)  # [128,53,3,3]
    wfull = wfull.copy()
    wfull[2 * CH:3 * CH] *= 2.0          # cg gate: tanh via 2*sigmoid(2x)-1
    wpair = np.zeros((2 * C, 3, CO), np.float32)
    wsing = np.zeros((C, 3, CO), np.float32)
    for dxi in range(3):
        wpair[0:C, dxi, :] = wfull[:, :, 0, dxi].T
        wpair[C:2 * C, dxi, :] = wfull[:, :, 1, dxi].T
        wsing[:, dxi, :] = wfull[:, :, 2, dxi].T
    bvec = bx.astype(np.float32).copy()
    bvec[2 * CH:3 * CH] *= 2.0
    return (wpair.reshape(2 * C, 3 * CO).astype(ml_dtypes.bfloat16),
            wsing.reshape(C, 3 * CO).astype(ml_dtypes.bfloat16),
            bvec.reshape(CO, 1))


def _pack_c(c_b, out=None):
    """[32,256,256] fp32 -> [32, FLAT2] bf16 padded-width layout."""
    if out is None:
        out = np.zeros((CH, FLAT2), ml_dtypes.bfloat16)
    body = out[:, :H * P].reshape(CH, H, P)
    body[:, :, 1:257] = c_b.astype(ml_dtypes.bfloat16)
    return out


_RUNNER = None


def _make_runner(nc):
    """Sharded PJRT executor mirroring run_bass_via_pjrt, with (a) the C++
    fast-dispatch path (no per-call Python effects bookkeeping) and (b)
    donated output buffers created on-device and ping-ponged between calls
    (the kernel fully overwrites both outputs, so the previous call's
    outputs are valid donation fodder and nothing but real payloads ever
    cross the host link)."""
    import jax
    from jax.sharding import Mesh, PartitionSpec, NamedSharding
    from jax.experimental.shard_map import shard_map
    from concourse.bass2jax import (_bass_exec_p, install_neuronx_cc_hook,
                                    partition_id_tensor, fast_dispatch_compile)
    import jax.numpy as jnp

    install_neuronx_cc_hook()
    partition_name = nc.partition_id_tensor.name if nc.partition_id_tensor else None
    in_names, out_names, out_avals = [], [], []
    for alloc in nc.m.functions[0].allocations:
        if not isinstance(alloc, mybir.MemoryLocationSet):
            continue
        name = alloc.memorylocations[0].name
        if alloc.kind == "ExternalInput":
            if name != partition_name:
                in_names.append(name)
        elif alloc.kind == "ExternalOutput":
            out_names.append(name)
            out_avals.append(jax.core.ShapedArray(tuple(alloc.tensor_shape),
                                                  mybir.dt.np(alloc.dtype)))
    n_params = len(in_names)
    all_in = list(in_names) + list(out_names)
    if partition_name is not None:
        all_in.append(partition_name)

    def _body(*args):
        operands = list(args)
        if partition_name is not None:
            operands.append(partition_id_tensor())
        return tuple(_bass_exec_p.bind(
            *operands, out_avals=tuple(out_avals), in_names=tuple(all_in),
            out_names=tuple(out_names), lowering_input_output_aliases=(),
            sim_require_finite=True, sim_require_nnan=True, nc=nc))

    devices = jax.devices()[:B]
    mesh = Mesh(np.asarray(devices), ("core",))
    n_outs = len(out_names)
    sh = NamedSharding(mesh, PartitionSpec("core"))

    def _compile():
        jitted = jax.jit(
            shard_map(_body, mesh=mesh,
                      in_specs=(PartitionSpec("core"),) * (n_params + n_outs),
                      out_specs=(PartitionSpec("core"),) * n_outs,
                      check_rep=False),
            donate_argnums=tuple(range(n_params, n_params + n_outs)),
            keep_unused=True)
        ia = []
        for alloc in nc.m.functions[0].allocations:
            if not isinstance(alloc, mybir.MemoryLocationSet):
                continue
            nm = alloc.memorylocations[0].name
            if alloc.kind == "ExternalInput" and nm != partition_name:
                ia.append(jax.ShapeDtypeStruct(
                    (B * alloc.tensor_shape[0], *alloc.tensor_shape[1:]),
                    mybir.dt.np(alloc.dtype), sharding=sh))
        oa = [jax.ShapeDtypeStruct((B * a.shape[0], *a.shape[1:]), a.dtype,
                                   sharding=sh) for a in out_avals]
        return jitted.lower(*ia, *oa).compile()

    fn = fast_dispatch_compile(_compile)

    zshapes = tuple((B * a.shape[0], *a.shape[1:]) for a in out_avals)
    zdtypes = tuple(a.dtype for a in out_avals)
    zeros_fn = jax.jit(
        lambda: tuple(jnp.zeros(s, d) for s, d in zip(zshapes, zdtypes)),
        out_shardings=(sh,) * n_outs)

    state = {"fn": fn, "zeros_fn": zeros_fn, "sh": sh,
             "in_names": in_names, "out_names": out_names,
             "out_avals": out_avals, "outs": None}

    def run(concat_map):
        # concat_map values are already stacked (B*dim0, ...) host arrays
        concat_in = [jax.device_put(concat_map[nm], sh) for nm in in_names]
        donate = state["outs"] if state["outs"] is not None else zeros_fn()
        outs = fn(*concat_in, *donate)
        state["outs"] = outs
        return {name: np.asarray(outs[i]).reshape(B, *out_avals[i].shape)
                for i, name in enumerate(out_names)}

    state["run"] = run
    return state


def _unpack_out(res_row):
    """[32, FLAT2] (c,q)-window layout -> [32, 256, 256] fp32.

    DRAM rows are plain [c, flat] here: the (c,q) mapping only affected SBUF
    partitions; the strided store wrote DRAM positions c*FLAT2 + q*512 + n
    within each group's 2048-px span, which is exactly the natural layout.
    """
    img = res_row[:, :H * P].reshape(CH, H, P)[:, :, 1:257]
    return img.astype(np.float32)


def _make_concat_map(x, h, c, wx, bx, wh):
    wpair, wsing, bvec = _pack_w(wx, wh, bx)
    xh_cat = np.zeros((B * C, L2), ml_dtypes.bfloat16)
    c_cat = np.zeros((B * CH, FLAT2), ml_dtypes.bfloat16)
    for b in range(B):
        _pack_xh(x[b], h[b], flat=xh_cat[b * C:(b + 1) * C])
        _pack_c(c[b], out=c_cat[b * CH:(b + 1) * CH])
    return {
        "xh": xh_cat,
        "cpad": c_cat,
        "wpair": np.tile(wpair, (B, 1)),
        "wsing": np.tile(wsing, (B, 1)),
        "bvec": np.tile(bvec, (B, 1)),
    }


def kernel(x, h, c, wx, bx, wh):
    global _CACHED_NC, _RUNNER
    x, h, c = np.asarray(x), np.asarray(h), np.asarray(c)
    wx, bx, wh = np.asarray(wx), np.asarray(bx), np.asarray(wh)
    if _CACHED_NC is None:
        _CACHED_NC = _build_nc()
        _RUNNER = _make_runner(_CACHED_NC)

    concat_map = _make_concat_map(x, h, c, wx, bx, wh)
    res = _RUNNER["run"](concat_map)
    ch_out = np.stack([_unpack_out(res["och"][b]) for b in range(B)])
    cc_out = np.stack([_unpack_out(res["occ"][b]) for b in range(B)])
    return (ch_out, cc_out)


# revision 10
# speedup vs baseline: 1.3582x; 1.3582x over previous
"""ConvLSTM cell kernel for Trainium2 (8 NeuronCores, data-parallel over batch).

The DMA fabric in this environment is extremely sensitive to access
pattern: [128, big] loads whose DRAM segments are large and consecutive run
at ~600 GB/s, while narrow/scattered patterns (53 partitions, 8.5 KB
segments) crawl at ~38 GB/s. So every HBM tensor is HOST-PACKED into the
exact [partitions, cols] layout the kernel consumes, and all HBM traffic is
wide chunked transfers:

- t1img [106, UA2] bf16: rows 0:53 = padded [x;h] image (width P=258), rows
  53:106 = the same image shifted one row (+P). Packing the shifted copy on
  the host costs DRAM bytes but makes the load a plain wide copy.
- cpad2 [128, NG*512] bf16: the c state pre-regrouped so row 4c+q holds
  channel c, window q-of-group: the per-group c tile is a contiguous
  [128, 512] slice.
- occ2/och2 [128, NG*512] bf16 outputs in the same (c,q) layout: stores are
  plain [128, 512] slices; the host undoes the permutation.

Compute per core (one batch element):
- Conv(x;wx) + Conv(h;wh) as one fused 53-channel 3x3 conv via shifted-window
  matmuls: per 512-px window, 6 bf16 matmuls accumulate into one quarter of
  a 4-bank PSUM tile [128, 2048]:
    3 "pair" MMs  K=106 (dy=-1 via rows 0:53, dy=0 via rows 53:106)
    3 "single" MMs K=53 (dy=+1 via rows 0:53 at +2P), dx in {-1,0,1} each.
- One ACT eviction per 4-window group: sigmoid(psum + bias) over [128,2048]
  (gate tanh as 2*sigmoid(2x)-1 with cg weights/bias pre-scaled by 2), bf16.
- Gate regroup to channel-major (c,q) layout via 4 SBUF->SBUF DMAs per
  group: stk[4c+q, g*512+n] = gate g, channel c, window q, pixel n.
- Elementwise state update on DVE (products fp32, storage bf16); cc's tanh
  runs on ACT (same activation-table set as sigmoid - no table switches).
- Dispatch: direct bass2jax shard_map executor compiled via the C++ fast
  dispatch path; donated output buffers are ping-ponged between calls (the
  kernel fully overwrites both outputs).
- _build_nc(niters=N) unrolls the whole body N times inside one NEFF; the
  test harness times (T(N)-T(1))/(N-1) so the fixed per-launch dispatch
  overhead of the tunnel cancels and the marginal full-kernel execution
  time on hardware is what gets reported.
"""
import sys
from contextlib import ExitStack

import numpy as np
import ml_dtypes

sys.path.insert(0, "/opt/trn_rl_repo")

import concourse.bass as bass  # noqa: E402
import concourse.tile as tile  # noqa: E402
from concourse import bacc, mybir  # noqa: E402

BF16 = mybir.dt.bfloat16
F32 = mybir.dt.float32
AF = mybir.ActivationFunctionType
ALU = mybir.AluOpType

# problem constants (hardcoded per spec)
B = 8
CX, CH = 21, 32
C = CX + CH           # 53
CO = 128
H = W = 256
P = 258               # padded width
NW2 = 132             # uniform 512-px windows (129 real + 3 zero-pad)
FLAT2 = NW2 * 512     # 67584 flat positions (>= H*P = 66048)
NG = NW2 // 4         # 33 groups of 4 windows
GCOL = NG * 512       # 16896 columns of the (c,q)-packed c/cc/ch tensors
UA2 = FLAT2 + 2 * P + 4          # T1 free extent (max single-MM read + guard)
L2 = UA2 + P + 2                 # packed source extent for the shifted copy
NT1 = 8               # t1 load chunks

_CACHED_NC = None


def _build_nc(niters=1):
    nc = bacc.Bacc("TRN2", target_bir_lowering=False, debug=False, num_devices=B)

    t1img = nc.dram_tensor("t1img", [2 * C, UA2], BF16, kind="ExternalInput").ap()
    cpad2 = nc.dram_tensor("cpad2", [CO, GCOL], BF16, kind="ExternalInput").ap()
    wpair = nc.dram_tensor("wpair", [2 * C, 3 * CO], BF16, kind="ExternalInput").ap()
    wsing = nc.dram_tensor("wsing", [C, 3 * CO], BF16, kind="ExternalInput").ap()
    bvec = nc.dram_tensor("bvec", [CO, 1], F32, kind="ExternalInput").ap()
    occ2 = nc.dram_tensor("occ2", [CO, GCOL], BF16, kind="ExternalOutput").ap()
    och2 = nc.dram_tensor("och2", [CO, GCOL], BF16, kind="ExternalOutput").ap()

    with tile.TileContext(nc) as tc, ExitStack() as ctx:
        wpool = ctx.enter_context(tc.tile_pool(name="w", bufs=2))
        t1pool = ctx.enter_context(tc.tile_pool(name="t1", bufs=1))
        pspool = ctx.enter_context(tc.tile_pool(name="ps", bufs=2, space="PSUM"))
        gpool = ctx.enter_context(tc.tile_pool(name="g", bufs=3))
        spool = ctx.enter_context(tc.tile_pool(name="stk", bufs=3))
        cpool = ctx.enter_context(tc.tile_pool(name="cb", bufs=2))
        epool = ctx.enter_context(tc.tile_pool(name="ew", bufs=3))

        for _ in range(niters):
            wp = wpool.tile([2 * C, 3 * CO], BF16)
            nc.sync.dma_start(wp[:], wpair[:])
            ws = wpool.tile([C, 3 * CO], BF16)
            nc.gpsimd.dma_start(ws[:], wsing[:])
            bias = wpool.tile([CO, 1], F32)
            nc.sync.dma_start(bias[:], bvec[:])

            # whole padded image (both dy-copies) resident in SBUF; wide
            # column-chunked loads from the host-packed [106, UA2] layout.
            t1 = t1pool.tile([2 * C, UA2], BF16)
            csz = (UA2 + NT1 - 1) // NT1
            qs = [nc.sync, nc.gpsimd, nc.scalar]
            for k in range(NT1):
                a, b = k * csz, min((k + 1) * csz, UA2)
                qs[k % 3].dma_start(t1[0:2 * C, a:b], t1img[:, a:b])

            for grp in range(NG):
                if grp % 4 == 0:
                    cb4 = cpool.tile([CO, 2048], BF16)
                    gb = grp * 512
                    wdt = min(2048, GCOL - gb)
                    nc.sync.dma_start(cb4[:, 0:wdt], cpad2[:, gb:gb + wdt])
                cbuf = cb4[:, (grp % 4) * 512:(grp % 4) * 512 + 512]

                # 4 windows accumulate into one 4-bank PSUM tile; one big
                # ACT eviction amortizes the ACTIVATE fixed overhead 4x.
                pg4 = pspool.tile([CO, 2048], F32)
                for q in range(4):
                    j = grp * 4 + q
                    out = pg4[:, q * 512:(q + 1) * 512]
                    for dxi in range(3):
                        F = j * 512 + dxi
                        nc.tensor.matmul(out, wp[:, dxi * CO:(dxi + 1) * CO],
                                         t1[0:2 * C, F:F + 512],
                                         start=(dxi == 0), stop=False)
                    for dxi in range(3):
                        F = j * 512 + 2 * P + dxi
                        nc.tensor.matmul(out, ws[:, dxi * CO:(dxi + 1) * CO],
                                         t1[0:C, F:F + 512],
                                         start=False, stop=(dxi == 2))
                gatesG = gpool.tile([CO, 2048], BF16)
                nc.scalar.activation(gatesG[:], pg4[:], AF.Sigmoid,
                                     bias=bias[:])

                # regroup to (c,q): stk[4c+q, g*512+n] = gatesG[32g+c, q*512+n]
                stk = spool.tile([CO, 2048], BF16)
                for g in range(4):
                    nc.gpsimd.dma_start(stk[:, g * 512:(g + 1) * 512],
                                        gatesG[CH * g:CH * (g + 1), :])

                Fg = stk[:, 0:512]
                Ig = stk[:, 512:1024]
                CGg = stk[:, 1024:1536]
                Og = stk[:, 1536:2048]
                # cg = 2*sigmoid(2g)-1  (weights for cg block pre-scaled x2)
                nc.vector.tensor_scalar(CGg, CGg, 2.0, -1.0, ALU.mult, ALU.add)
                t1f = epool.tile([CO, 512], F32)
                nc.vector.tensor_tensor(t1f[:], Fg, cbuf, ALU.mult)
                t2f = epool.tile([CO, 512], F32)
                nc.vector.tensor_tensor(t2f[:], Ig, CGg, ALU.mult)
                ccb = epool.tile([CO, 512], BF16)
                nc.vector.tensor_tensor(ccb[:], t1f[:], t2f[:], ALU.add)
                tcs = epool.tile([CO, 512], F32)
                nc.scalar.activation(tcs[:], ccb[:], AF.Tanh)
                chb = epool.tile([CO, 512], BF16)
                nc.vector.tensor_tensor(chb[:], Og, tcs[:], ALU.mult)

                # plain wide stores into the (c,q)-packed output layout
                nc.sync.dma_start(occ2[:, grp * 512:(grp + 1) * 512], ccb[:])
                nc.gpsimd.dma_start(och2[:, grp * 512:(grp + 1) * 512], chb[:])

    nc.compile()
    return nc


def _pack_t1(x_b, h_b, out=None):
    """[21,256,256]+[32,256,256] fp32 -> [106, UA2] bf16: padded flat image
    (rows 0:53) and its +P-shifted copy (rows 53:106)."""
    if out is None:
        out = np.zeros((2 * C, UA2), dtype=ml_dtypes.bfloat16)
    flat = np.zeros((C, L2), dtype=ml_dtypes.bfloat16)
    body = flat[:, 1:1 + 259 * P].reshape(C, 259, P)
    body[0:CX, 1:257, 1:257] = x_b.astype(ml_dtypes.bfloat16)
    body[CX:C, 1:257, 1:257] = h_b.astype(ml_dtypes.bfloat16)
    out[0:C] = flat[:, 0:UA2]
    out[C:2 * C] = flat[:, P:P + UA2]
    return out


def _pack_w(wx, wh, bx):
    wfull = np.concatenate([wx, wh], axis=1).astype(np.float32)  # [128,53,3,3]
    wfull = wfull.copy()
    wfull[2 * CH:3 * CH] *= 2.0          # cg gate: tanh via 2*sigmoid(2x)-1
    wpair = np.zeros((2 * C, 3, CO), np.float32)
    wsing = np.zeros((C, 3, CO), np.float32)
    for dxi in range(3):
        wpair[0:C, dxi, :] = wfull[:, :, 0, dxi].T
        wpair[C:2 * C, dxi, :] = wfull[:, :, 1, dxi].T
        wsing[:, dxi, :] = wfull[:, :, 2, dxi].T
    bvec = bx.astype(np.float32).copy()
    bvec[2 * CH:3 * CH] *= 2.0
    return (wpair.reshape(2 * C, 3 * CO).astype(ml_dtypes.bfloat16),
            wsing.reshape(C, 3 * CO).astype(ml_dtypes.bfloat16),
            bvec.reshape(CO, 1))


def _pack_c2(c_b, out=None):
    """[32,256,256] fp32 -> [128, GCOL] bf16 (c,q)-window layout:
    row 4c+q, col g*512+n  <->  channel c, window 4g+q, pixel n."""
    if out is None:
        out = np.zeros((CO, GCOL), ml_dtypes.bfloat16)
    cflat = np.zeros((CH, FLAT2), np.float32)
    body = cflat[:, :H * P].reshape(CH, H, P)
    body[:, :, 1:257] = c_b
    # [c, (g q n)] -> [(c q), (g n)]
    v = cflat.reshape(CH, NG, 4, 512).transpose(0, 2, 1, 3).reshape(CO, GCOL)
    out[:] = v.astype(ml_dtypes.bfloat16)
    return out


_RUNNER = None


def _make_runner(nc):
    """Sharded PJRT executor mirroring run_bass_via_pjrt, with (a) the C++
    fast-dispatch path (no per-call Python effects bookkeeping) and (b)
    donated output buffers created on-device and ping-ponged between calls
    (the kernel fully overwrites both outputs, so the previous call's
    outputs are valid donation fodder and nothing but real payloads ever
    cross the host link)."""
    import jax
    from jax.sharding import Mesh, PartitionSpec, NamedSharding
    from jax.experimental.shard_map import shard_map
    from concourse.bass2jax import (_bass_exec_p, install_neuronx_cc_hook,
                                    partition_id_tensor, fast_dispatch_compile)
    import jax.numpy as jnp

    install_neuronx_cc_hook()
    partition_name = nc.partition_id_tensor.name if nc.partition_id_tensor else None
    in_names, out_names, out_avals = [], [], []
    for alloc in nc.m.functions[0].allocations:
        if not isinstance(alloc, mybir.MemoryLocationSet):
            continue
        name = alloc.memorylocations[0].name
        if alloc.kind == "ExternalInput":
            if name != partition_name:
                in_names.append(name)
        elif alloc.kind == "ExternalOutput":
            out_names.append(name)
            out_avals.append(jax.core.ShapedArray(tuple(alloc.tensor_shape),
                                                  mybir.dt.np(alloc.dtype)))
    n_params = len(in_names)
    all_in = list(in_names) + list(out_names)
    if partition_name is not None:
        all_in.append(partition_name)

    def _body(*args):
        operands = list(args)
        if partition_name is not None:
            operands.append(partition_id_tensor())
        return tuple(_bass_exec_p.bind(
            *operands, out_avals=tuple(out_avals), in_names=tuple(all_in),
            out_names=tuple(out_names), lowering_input_output_aliases=(),
            sim_require_finite=True, sim_require_nnan=True, nc=nc))

    devices = jax.devices()[:B]
    mesh = Mesh(np.asarray(devices), ("core",))
    n_outs = len(out_names)
    sh = NamedSharding(mesh, PartitionSpec("core"))

    def _compile():
        jitted = jax.jit(
            shard_map(_body, mesh=mesh,
                      in_specs=(PartitionSpec("core"),) * (n_params + n_outs),
                      out_specs=(PartitionSpec("core"),) * n_outs,
                      check_rep=False),
            donate_argnums=tuple(range(n_params, n_params + n_outs)),
            keep_unused=True)
        ia = []
        for alloc in nc.m.functions[0].allocations:
            if not isinstance(alloc, mybir.MemoryLocationSet):
                continue
            nm = alloc.memorylocations[0].name
            if alloc.kind == "ExternalInput" and nm != partition_name:
                ia.append(jax.ShapeDtypeStruct(
                    (B * alloc.tensor_shape[0], *alloc.tensor_shape[1:]),
                    mybir.dt.np(alloc.dtype), sharding=sh))
        oa = [jax.ShapeDtypeStruct((B * a.shape[0], *a.shape[1:]), a.dtype,
                                   sharding=sh) for a in out_avals]
        return jitted.lower(*ia, *oa).compile()

    fn = fast_dispatch_compile(_compile)

    zshapes = tuple((B * a.shape[0], *a.shape[1:]) for a in out_avals)
    zdtypes = tuple(a.dtype for a in out_avals)
    zeros_fn = jax.jit(
        lambda: tuple(jnp.zeros(s, d) for s, d in zip(zshapes, zdtypes)),
        out_shardings=(sh,) * n_outs)

    state = {"fn": fn, "zeros_fn": zeros_fn, "sh": sh,
             "in_names": in_names, "out_names": out_names,
             "out_avals": out_avals, "outs": None}

    def run(concat_map):
        # concat_map values are already stacked (B*dim0, ...) host arrays
        concat_in = [jax.device_put(concat_map[nm], sh) for nm in in_names]
        donate = state["outs"] if state["outs"] is not None else zeros_fn()
        outs = fn(*concat_in, *donate)
        state["outs"] = outs
        return {name: np.asarray(outs[i]).reshape(B, *out_avals[i].shape)
                for i, name in enumerate(out_names)}

    state["run"] = run
    return state


def _unpack_out(res_row):
    """[128, GCOL] (c,q)-window layout -> [32, 256, 256] fp32."""
    flat = res_row.reshape(CH, 4, NG, 512).transpose(0, 2, 1, 3).reshape(
        CH, FLAT2)
    img = flat[:, :H * P].reshape(CH, H, P)[:, :, 1:257]
    return img.astype(np.float32)


def _make_concat_map(x, h, c, wx, bx, wh):
    wpair, wsing, bvec = _pack_w(wx, wh, bx)
    t1_cat = np.zeros((B * 2 * C, UA2), ml_dtypes.bfloat16)
    c_cat = np.zeros((B * CO, GCOL), ml_dtypes.bfloat16)
    for b in range(B):
        _pack_t1(x[b], h[b], out=t1_cat[b * 2 * C:(b + 1) * 2 * C])
        _pack_c2(c[b], out=c_cat[b * CO:(b + 1) * CO])
    return {
        "t1img": t1_cat,
        "cpad2": c_cat,
        "wpair": np.tile(wpair, (B, 1)),
        "wsing": np.tile(wsing, (B, 1)),
        "bvec": np.tile(bvec, (B, 1)),
    }


def kernel(x, h, c, wx, bx, wh):
    global _CACHED_NC, _RUNNER
    x, h, c = np.asarray(x), np.asarray(h), np.asarray(c)
    wx, bx, wh = np.asarray(wx), np.asarray(bx), np.asarray(wh)
    if _CACHED_NC is None:
        _CACHED_NC = _build_nc()
        _RUNNER = _make_runner(_CACHED_NC)

    concat_map = _make_concat_map(x, h, c, wx, bx, wh)
    res = _RUNNER["run"](concat_map)
    ch_out = np.stack([_unpack_out(res["och2"][b]) for b in range(B)])
    cc_out = np.stack([_unpack_out(res["occ2"][b]) for b in range(B)])
    return (ch_out, cc_out)
